# revision 4
# baseline (speedup 1.0000x reference)
"""TRN2 Bass kernel for nn_CustomMLPLayer_10806137716759.

Computes, for x [1, S, F] f32 and W [D, F] f32:
  true_value [1, S, D] = x @ W.T
  neuron_mask [F] bool = counts > floor(mean(counts) * 0.95),
    counts[f] = #{s : x[0, s, f] > 0}

Strategy: tensor-parallel over D across 8 NeuronCores (each core owns a
512-column shard of W and all of x).  The contraction axis F is tiled
into 8 "eighths" of 11 k-tiles (128 each, F padded 11008 -> 11264); W
eighths stream through SBUF while partial [s, d] outputs accumulate in
PSUM and spill to an SBUF partials buffer between eighths.  Matmuls run
in float32r (TF32-like, 1 cycle/row at N=512); operands are rounded to
f32r on DVE (required by the BIR verifier), optionally split hi/lo for
near-fp32 accuracy (SPLIT=3).  Per-neuron activation counts are computed
on-device (is_gt + reduce over the token axis) and finalized on host.
"""
import sys
if '/opt/trn_rl_repo' not in sys.path:
    sys.path.insert(0, '/opt/trn_rl_repo')

import numpy as np

S, F, D = 2048, 11008, 4096
NCORES = 8
DSH = D // NCORES        # 512 output columns per core
KT = 88                  # padded k tiles (F padded to 11264)
FP = KT * 128
NQ = 11                  # k chunks streamed through SBUF
KQ = KT // NQ            # 8 k-tiles per chunk
NS = S // 128            # 16 s-strips
SPLIT = 3                # 1 = single f32r pass (~1.5e-4 rel err),
                         # 3 = hi/lo split (~1e-7 rel err, 3x PE work)

_CACHE = {}


def _build_program():
    import concourse.bass as bass
    import concourse.tile as tile
    from concourse import mybir, bacc

    f32 = mybir.dt.float32
    f32r = mybir.dt.float32r
    AX = mybir.AxisListType
    OP = mybir.AluOpType

    nc = bacc.Bacc("TRN2", target_bir_lowering=False, debug=False)
    xs = nc.dram_tensor("xs", [NQ, NS, 128, KQ, 128], f32, kind="ExternalInput")
    wt = nc.dram_tensor("wt", [NQ, 128, KQ, DSH], f32, kind="ExternalInput")
    out = nc.dram_tensor("out", [NS, 128, DSH], f32, kind="ExternalOutput")
    cnt = nc.dram_tensor("cnt", [128, KT], f32, kind="ExternalOutput")

    with tile.TileContext(nc) as tc:
        with tc.tile_pool(name="wp", bufs=2) as wp, \
             tc.tile_pool(name="whp", bufs=2) as whp, \
             tc.tile_pool(name="wlp", bufs=2) as wlp, \
             tc.tile_pool(name="xp", bufs=3) as xp, \
             tc.tile_pool(name="xhp", bufs=3) as xhp, \
             tc.tile_pool(name="xlp", bufs=3) as xlp, \
             tc.tile_pool(name="pt", bufs=1) as pt, \
             tc.tile_pool(name="op_", bufs=2) as op_, \
             tc.tile_pool(name="scp", bufs=2) as scp, \
             tc.tile_pool(name="rdp", bufs=2) as rdp, \
             tc.tile_pool(name="ps", bufs=4, space="PSUM") as ps:

            partials = pt.tile([128, NS, DSH], f32)
            counts = pt.tile([128, KT], f32)
            nc.vector.memset(counts[:], 0.0)

            for h in range(NQ):
                w_raw = wp.tile([128, KQ, DSH], f32)
                nc.sync.dma_start(out=w_raw[:], in_=wt[h])
                w_hi = whp.tile([128, KQ, DSH], f32r)
                nc.vector.tensor_copy(w_hi[:], w_raw[:])
                if SPLIT == 1:
                    w_ops = [w_hi]
                else:
                    w_lo = wlp.tile([128, KQ, DSH], f32r)
                    nc.vector.tensor_sub(w_lo[:], w_raw[:],
                                         w_hi[:].bitcast(f32))
                    w_ops = [w_hi, w_lo]

                for n in range(NS):
                    x_raw = xp.tile([128, KQ, 128], f32)
                    nc.sync.dma_start(out=x_raw[:], in_=xs[h, n])
                    x_hi = xhp.tile([128, KQ, 128], f32r)
                    nc.vector.tensor_copy(x_hi[:], x_raw[:])
                    x_stat = x_hi
                    if SPLIT == 1:
                        pairs = [(x_hi, w_hi)]
                    else:
                        x_lo = xlp.tile([128, KQ, 128], f32r)
                        nc.vector.tensor_sub(x_lo[:], x_raw[:],
                                             x_hi[:].bitcast(f32))
                        # hi*hi + hi*lo + lo*hi  (lo*lo dropped)
                        pairs = [(x_hi, w_ops[0]), (x_hi, w_ops[1]),
                                 (x_lo, w_ops[0])]

                    psum = ps.tile([128, DSH], f32)
                    n_mm = len(pairs) * KQ
                    i = 0
                    for j2 in range(KQ):
                        for (xt, wtile) in pairs:
                            nc.tensor.matmul(
                                psum[:],
                                xt[:, j2, :].bitcast(f32r),
                                wtile[:, j2, :].bitcast(f32r),
                                start=(i == 0), stop=(i == n_mm - 1))
                            i += 1

                    if h == 0:
                        nc.vector.tensor_copy(partials[:, n, :], psum[:])
                    elif h < NQ - 1:
                        nc.vector.tensor_add(partials[:, n, :],
                                             partials[:, n, :], psum[:])
                    else:
                        ot = op_.tile([128, DSH], f32)
                        nc.vector.tensor_add(ot[:], partials[:, n, :], psum[:])
                        nc.sync.dma_start(out=out[n], in_=ot[:])

                    # per-neuron activation counts for this eighth's k range
                    sc = scp.tile([128, KQ, 128], f32)
                    nc.vector.tensor_scalar(sc[:], x_stat[:].bitcast(f32),
                                            0.0, None, OP.is_gt)
                    rd = rdp.tile([128, KQ], f32)
                    nc.vector.tensor_reduce(rd[:], sc[:], axis=AX.X, op=OP.add)
                    nc.vector.tensor_add(counts[:, h * KQ:(h + 1) * KQ],
                                         counts[:, h * KQ:(h + 1) * KQ], rd[:])

            nc.sync.dma_start(out=cnt[:], in_=counts[:])

    nc.compile()
    return nc


def _prep_inputs(x, W):
    """Host-side relayout: both operands contraction-major, pre-tiled."""
    x2 = np.ascontiguousarray(x.reshape(S, F), dtype=np.float32)
    xpad = np.zeros((S, FP), dtype=np.float32)
    xpad[:, :F] = x2
    # [n, j, h, j2, p] -> [h, n, p, j2, j]
    xs = np.ascontiguousarray(
        xpad.reshape(NS, 128, NQ, KQ, 128).transpose(2, 0, 4, 3, 1))

    Wp = np.zeros((D, FP), dtype=np.float32)
    Wp[:, :F] = W
    wts = []
    for c in range(NCORES):
        Wc = Wp[c * DSH:(c + 1) * DSH]          # [DSH, FP]
        # [d, h, j2, p] -> [h, p, j2, d]
        wts.append(np.ascontiguousarray(
            Wc.reshape(DSH, NQ, KQ, 128).transpose(1, 3, 2, 0)))
    return xs, wts


def _run(nc, xs, wts, trace=False):
    from concourse.bass_utils import run_bass_kernel_spmd
    in_maps = [{"xs": xs, "wt": wts[c]} for c in range(NCORES)]
    return run_bass_kernel_spmd(nc, in_maps, core_ids=list(range(NCORES)),
                                trace=trace)


def _assemble(res):
    outs = [res.results[c]["out"].reshape(S, DSH) for c in range(NCORES)]
    true_value = np.concatenate(outs, axis=1).reshape(1, S, D)

    cntm = res.results[0]["cnt"]                 # [128, KT]
    counts = np.ascontiguousarray(cntm.T).reshape(KT * 128)[:F]
    mean = counts.astype(np.float32).mean(dtype=np.float32)
    cutoff = np.floor(mean * np.float32(0.95))
    neuron_mask = counts > cutoff
    return true_value, neuron_mask


def kernel(x, W):
    if "nc" not in _CACHE:
        _CACHE["nc"] = _build_program()
    nc = _CACHE["nc"]
    xs, wts = _prep_inputs(np.asarray(x), np.asarray(W))
    res = _run(nc, xs, wts)
    return _assemble(res)
